# revision 30
# baseline (speedup 1.0000x reference)
"""Trainium2 Bass kernel for nn_Attention_15676630631260 (sparse_attention).

reference:
  q = x @ Wq.T + bq ; k = x @ Wk.T + bk ; v = x @ Wv.T + bv        (per batch)
  scores = sigmoid(q @ k.T / sqrt(P))                               [B,S,S]
  out[b,i,j,:] = tril(i,j) * scores[b,i,j] * v[b,j,:]               [B,S,S,P]

B=2, S=512, D=256, P=128.  Output is 256 MB; the causal mask zeroes the
j>i region.  run_bass_kernel_spmd pre-zeroes ExternalOutput buffers
(donated zero buffers under PJRT), so the kernel only writes the j<=i
region — at 128-column tile granularity per row: row i writes j-tiles
0..i//128 (the partial diagonal tile is zeroed exactly via a
host-supplied mask).

Sharding (8 cores, one NEFF, SPMD): core c -> batch b=c//4, quarter
k=c%4.  Rows are assigned as 16-row blocks paired (m, 31-m) so every
core's multiset of per-row written-tile-counts is {1,1,2,2,3,3,4,4} per
block pair -> identical instruction stream on every core, only input
data differs, and every core writes exactly 20 MB of its 32 MB shard.

Per-core device program (all matmuls fp32r: single-pass, ~tf32-grade):
  Q^T[p,i]; K^T/V^T/scores per j-tile.  Tiles 0 and 1 take a "narrow"
  fast path whose inputs come from ONE packed critical-input DMA (one
  first-byte latency); tiles 2-3 take a wide N=256 path.  scores are
  computed as [i,j], sigmoid'd on ACT, PE-transposed to [j,i], masked
  on DVE.  V^T gets its bias on the PSUM->SBUF copy (per-partition
  tensor_scalar add) and is PE-transposed to V[s,p].
  Output rows are produced as [j_partition, (jt, i, p)] slabs:
  broadcast row-scaling of V by score columns, batched 8 rows per DVE
  tensor_tensor (stride-0 broadcast APs), with a spread subset done as
  per-row activation-scale ops on ACT to balance the engines; then
  batched HWDGE DMAs ([j, jt, (i p)] — 4 KB contiguous runs per
  partition) into the [j, i_local, p]-layout local output shard.
Classes stream in order 0, 1, 3, 2 (fast ramp, small tail).
"""

import os
import sys

import numpy as np

for _p in ("/root/.axon_site/_ro/trn_rl_repo", "/opt/trn_rl_repo"):
    if _p not in sys.path and os.path.isdir(_p):
        sys.path.append(_p)

import concourse.bass as bass
import concourse.mybir as mybir
from concourse.tile import TileContext
from concourse.masks import make_identity
from concourse import bass_utils

F32 = mybir.dt.float32
F32R = mybir.dt.float32r
B, S, D, P = 2, 512, 256, 128
NCORES = 8
GROUP = 8           # output rows per DMA group
NGROUPS = 128 // GROUP
INV_SQRT_P = float(1.0 / np.sqrt(np.float32(P)))
# packed critical-input column offsets:
# wq(2x128) wk(2x128) wv(2x128) xq(2x128) xt0(2x128) xt1(2x128) b3(3) mk0(128) mk1(128)
OFF_XQ = 6 * 128
OFF_XT0 = 8 * 128
OFF_XT1 = 10 * 128
OFF_B3 = 12 * 128
OFF_MK0 = 12 * 128 + 3
OFF_MK1 = 12 * 128 + 3 + 128
CRIT_COLS = 12 * 128 + 3 + 2 * 128
# (g, jt) group-tile indices (in emission order) produced on ACT instead
# of DVE: ACT per-row ops cost ~0.49us vs DVE ~0.152us/row batched, so
# ACT gets ~10/40 of the work, spread through the schedule.
ACT_SET = frozenset({1, 6, 11, 16, 21, 26, 31, 36, 38})


def _blocks16(k: int) -> list[int]:
    # 16-row blocks (32 per batch) for quarter k, ordered so written
    # j-tile count ti=m//8 ascends: [0,0,1,1,2,2,3,3]
    return [k, k + 4, k + 8, k + 12, 19 - k, 23 - k, 27 - k, 31 - k]


def _rows_sel(k: int) -> np.ndarray:
    return np.concatenate([np.arange(16 * m, 16 * m + 16) for m in _blocks16(k)])


def _build_nc() -> bass.Bass:
    nc = bass.Bass(trn_type="TRN2")

    xt = nc.dram_tensor("xt", [D, S], F32R, kind="ExternalInput")   # x[b].T
    mk = nc.dram_tensor("mk", [4, 128, 128], F32, kind="ExternalInput")
    crit = nc.dram_tensor("crit", [128, CRIT_COLS], F32R, kind="ExternalInput")
    # local output layout [j, i_local, p]: per-DMA-partition runs are
    # (i,p)-contiguous (4 KB per 8-row group) instead of 512 B
    out = nc.dram_tensor("out", [S, 128, P], F32, kind="ExternalOutput")

    with TileContext(nc) as tc:
        with (
            tc.tile_pool(name="const", bufs=1) as cpool,
            tc.tile_pool(name="psA", bufs=1, space="PSUM") as psA,
            tc.tile_pool(name="psW", bufs=2, space="PSUM") as psW,
            tc.tile_pool(name="psB", bufs=2, space="PSUM") as psB,
            tc.tile_pool(name="slab", bufs=3) as spool,
        ):
            # ---- input loads ----
            # ONE packed DMA (sync ring) delivers everything tiles 0-1
            # need; bulk tails (x tiles 2-3, masks 2-3) on the GpSimd
            # SWDGE ring.  Sync ring then belongs to the output DMAs.
            xt_r = xt.rearrange("(c p) s -> p c s", p=128)     # [128, 2, 512]

            crit_sb = cpool.tile([128, CRIT_COLS], F32R, tag="crit")
            nc.sync.dma_start(crit_sb[:], crit[:])

            xtR_sb = cpool.tile([128, 2 * 256], F32R, tag="xtR")
            nc.gpsimd.dma_start(
                xtR_sb[:].rearrange("q (c m) -> q c m", c=2),
                xt_r[:, :, 256:512],
            )
            mkR_sb = cpool.tile([128, 2 * 128], F32, tag="mkR")
            nc.gpsimd.dma_start(
                mkR_sb[:].rearrange("q (t i) -> q t i", t=2),
                mk.rearrange("t j i -> j t i")[:, 2:4, :],
            )

            def cslice(idx, n=128):
                return crit_sb[:, idx : idx + n]

            bq_c = cslice(OFF_B3, 1).bitcast(F32)
            bk_c = cslice(OFF_B3 + 1, 1).bitcast(F32)
            bv_c = cslice(OFF_B3 + 2, 1).bitcast(F32)

            def wqk(which, c):  # 0=q, 1=k
                return cslice((which * 2 + c) * P, P)

            def wv(c):
                return cslice((4 + c) * P, P)

            identity = cpool.tile([128, 128], F32, tag="ident")
            nc.vector.memset(identity[:], 0.0)
            make_identity(nc, identity[:], nomemset=True)

            # Q^T [p, i]
            qt_ps = psA.tile([128, 128], F32, tag="qtps")
            nc.tensor.matmul(qt_ps[:], wqk(0, 0), cslice(OFF_XQ), start=True, stop=False)
            nc.tensor.matmul(qt_ps[:], wqk(0, 1), cslice(OFF_XQ + 128), start=False, stop=True)
            qt_sb = cpool.tile([128, 128], F32R, tag="qt")
            nc.vector.tensor_scalar_add(qt_sb[:], qt_ps[:], bq_c)

            kt_t = [None] * 4
            v_t = [None] * 4
            stm_t = [None] * 4

            def make_narrow(jt: int, xt_off: int, mk_off: int):
                ktp = psB.tile([128, 128], F32, tag="proj", name=f"ktp{jt}")
                nc.tensor.matmul(ktp[:], wqk(1, 0), cslice(xt_off), start=True, stop=False)
                nc.tensor.matmul(ktp[:], wqk(1, 1), cslice(xt_off + 128), start=False, stop=True)
                ktile = cpool.tile([128, 128], F32R, tag=f"kt{jt}", name=f"kt{jt}")
                nc.vector.tensor_scalar_add(ktile[:], ktp[:], bk_c)
                kt_t[jt] = ktile

                sp = psB.tile([128, 128], F32, tag="proj", name=f"sps{jt}")
                nc.tensor.matmul(sp[:], qt_sb[:], ktile[:], start=True, stop=True)
                st = cpool.tile([128, 128], F32, tag=f"st{jt}", name=f"st{jt}")
                nc.scalar.activation(
                    st[:], sp[:], mybir.ActivationFunctionType.Sigmoid,
                    scale=INV_SQRT_P,
                )
                stp = psB.tile([128, 128], F32, tag="tp", name=f"stp{jt}")
                nc.tensor.transpose(stp[:], st[:], identity[:])
                stm = cpool.tile([128, 128], F32, tag=f"stm{jt}", name=f"stm{jt}")
                nc.vector.tensor_mul(stm[:], stp[:], cslice(mk_off).bitcast(F32))
                stm_t[jt] = stm

                vtp = psB.tile([128, 128], F32, tag="proj", name=f"vtp{jt}")
                nc.tensor.matmul(vtp[:], wv(0), cslice(xt_off), start=True, stop=False)
                nc.tensor.matmul(vtp[:], wv(1), cslice(xt_off + 128), start=False, stop=True)
                vT = cpool.tile([128, 128], F32, tag=f"vT{jt}", name=f"vT{jt}")
                nc.scalar.add(vT[:], vtp[:], add=bv_c)
                vp = psB.tile([128, 128], F32, tag="tp", name=f"vp{jt}")
                nc.tensor.transpose(vp[:], vT[:], identity[:])
                vt = cpool.tile([128, P], F32, tag=f"v{jt}", name=f"v{jt}")
                nc.scalar.copy(vt[:], vp[:])
                v_t[jt] = vt

            def make_wide():
                # tiles 2-3 in one N=256 fp32r pass each
                ktpR = psW.tile([128, 256], F32, tag="wide", name="ktpR")
                nc.tensor.matmul(ktpR[:], wqk(1, 0), xtR_sb[:, 0:256], start=True, stop=False)
                nc.tensor.matmul(ktpR[:], wqk(1, 1), xtR_sb[:, 256:512], start=False, stop=True)
                ktR = cpool.tile([128, 256], F32R, tag="ktR")
                nc.vector.tensor_scalar_add(ktR[:], ktpR[:], bk_c)

                spR = psW.tile([128, 256], F32, tag="wide", name="spR")
                nc.tensor.matmul(spR[:], qt_sb[:], ktR[:], start=True, stop=True)
                stR = cpool.tile([128, 256], F32, tag="stR")
                nc.scalar.activation(
                    stR[:], spR[:], mybir.ActivationFunctionType.Sigmoid,
                    scale=INV_SQRT_P,
                )
                vtpR = psW.tile([128, 256], F32, tag="wide", name="vtpR")
                nc.tensor.matmul(vtpR[:], wv(0), xtR_sb[:, 0:256], start=True, stop=False)
                nc.tensor.matmul(vtpR[:], wv(1), xtR_sb[:, 256:512], start=False, stop=True)
                vTR = cpool.tile([128, 256], F32, tag="vTR")
                nc.scalar.add(vTR[:], vtpR[:], add=bv_c)

                for jt in (2, 3):
                    c = jt - 2
                    stp = psB.tile([128, 128], F32, tag="tp", name=f"stp{jt}")
                    nc.tensor.transpose(stp[:], stR[:, c * 128 : (c + 1) * 128], identity[:])
                    stm = cpool.tile([128, 128], F32, tag=f"stm{jt}", name=f"stm{jt}")
                    nc.vector.tensor_mul(
                        stm[:], stp[:], mkR_sb[:, c * 128 : (c + 1) * 128]
                    )
                    stm_t[jt] = stm
                    vp = psB.tile([128, 128], F32, tag="tp", name=f"vp{jt}")
                    nc.tensor.transpose(vp[:], vTR[:, c * 128 : (c + 1) * 128], identity[:])
                    vt = cpool.tile([128, P], F32, tag=f"v{jt}", name=f"v{jt}")
                    nc.scalar.copy(vt[:], vp[:])
                    v_t[jt] = vt

            # ---- output slab stage ----
            out_r = out.rearrange("(t j) i p -> j t (i p)", j=128)  # [128,4,16384]
            gt_counter = [0]

            def emit_group(g: int):
                L = g // (NGROUPS // 4) + 1
                slab = spool.tile(
                    [128, L * GROUP * 128], F32, tag=f"slab{L}", name=f"slab_g{g}"
                )
                for jt in range(L):
                    dst3 = slab[
                        :, jt * GROUP * 128 : (jt + 1) * GROUP * 128
                    ].rearrange("q (i p) -> q i p", i=GROUP)
                    eng = "A" if gt_counter[0] in ACT_SET else "D"
                    gt_counter[0] += 1
                    if eng == "D":
                        v3 = v_t[jt][:].unsqueeze(1).broadcast_to([128, GROUP, 128])
                        s3 = (
                            stm_t[jt][:, g * GROUP : (g + 1) * GROUP]
                            .unsqueeze(2)
                            .broadcast_to([128, GROUP, 128])
                        )
                        nc.vector.tensor_mul(dst3, v3, s3)
                    else:
                        for ii in range(GROUP):
                            li = g * GROUP + ii
                            nc.scalar.mul(
                                dst3[:, ii, :],
                                v_t[jt][:],
                                mul=stm_t[jt][:, li : li + 1],
                            )
                nc.sync.dma_start(
                    out_r[:, 0:L, GROUP * 128 * g : GROUP * 128 * (g + 1)],
                    slab[:].rearrange("q (t ip) -> q t ip", t=L),
                )

            make_narrow(0, OFF_XT0, OFF_MK0)
            for g in range(0, 4):      # class 0 (tile 0 only)
                emit_group(g)
            make_narrow(1, OFF_XT1, OFF_MK1)
            for g in range(4, 8):      # class 1 (tiles 0-1)
                emit_group(g)
            make_wide()
            for g in range(12, 16):    # class 3
                emit_group(g)
            for g in range(8, 12):     # class 2
                emit_group(g)

    _split_multi_waits(nc)
    return nc


def _split_multi_waits(nc):
    """This toolchain's walrus accepts at most one sync wait per
    instruction; split extras into single-wait NoOps just before the
    instruction on the same engine queue (waits are ANDed preconditions,
    executed in order on the engine's queue — semantically identical)."""
    for fn in nc.m.functions:
        for blk in fn.blocks:
            insts = blk.instructions
            i = 0
            while i < len(insts):
                inst = insts[i]
                si = getattr(inst, "sync_info", None)
                if si is not None and si.on_wait is not None and len(si.on_wait) > 1:
                    waits = list(si.on_wait)
                    nops = [
                        mybir.InstNoOp(
                            name=nc.get_next_instruction_name(),
                            engine=inst.engine,
                            sync_info=mybir.SyncInfo(on_wait=[w], on_update=[]),
                            bass_nofuse=True,
                        )
                        for w in waits[:-1]
                    ]
                    si.on_wait = [waits[-1]]
                    insts[i:i] = nops
                    i += len(nops)
                i += 1


_NC_CACHE = None


def _get_nc():
    global _NC_CACHE
    if _NC_CACHE is None:
        _NC_CACHE = _build_nc()
    return _NC_CACHE


def _in_maps(x_set, Wq, bq, Wk, bk, Wv, bv):
    xts = [
        np.ascontiguousarray(x_set[b].T).astype(np.float32, copy=False)
        for b in range(B)
    ]
    wqT, wkT, wvT = Wq.T, Wk.T, Wv.T
    b3c = np.stack([bq, bk, bv], axis=1).astype(np.float32)
    jj = np.arange(128)
    maps = []
    for c in range(NCORES):
        b, k = divmod(c, 4)
        rows = _rows_sel(k)
        xtT = xts[b]
        xqT = xtT[:, rows]
        mask = np.empty((4, 128, 128), np.float32)
        for jt in range(4):
            mask[jt] = ((jt * 128 + jj)[:, None] <= rows[None, :]).astype(np.float32)
        crit = np.concatenate(
            [
                wqT[0:128], wqT[128:256],
                wkT[0:128], wkT[128:256],
                wvT[0:128], wvT[128:256],
                xqT[0:128], xqT[128:256],
                xtT[0:128, 0:128], xtT[128:256, 0:128],
                xtT[0:128, 128:256], xtT[128:256, 128:256],
                b3c, mask[0], mask[1],
            ],
            axis=1,
        ).astype(np.float32, copy=False)
        maps.append(
            {
                "xt": xtT,
                "mk": mask,
                "crit": np.ascontiguousarray(crit),
            }
        )
    return maps


def run(x_set, Wq, bq, Wk, bk, Wv, bv, **spmd_kwargs):
    nc = _get_nc()
    in_maps = _in_maps(x_set, Wq, bq, Wk, bk, Wv, bv)
    res = bass_utils.run_bass_kernel_spmd(
        nc, in_maps, core_ids=list(range(NCORES)), **spmd_kwargs
    )
    full = np.zeros((B, S, S, P), np.float32)
    for c in range(NCORES):
        b, k = divmod(c, 4)
        # core output is [j, i_local, p] -> scatter as [i_local, j, p]
        full[b, _rows_sel(k)] = res.results[c]["out"].transpose(1, 0, 2)
    return full, res


def kernel(x_set, Wq, bq, Wk, bk, Wv, bv):
    full, _ = run(x_set, Wq, bq, Wk, bk, Wv, bv)
    return full
